# revision 29
# baseline (speedup 1.0000x reference)
"""Bass/Trainium2 kernel for nn_Attn (dot+affect attention over encoder outputs).

Computation (per batch b):
  e[b, l] = h[b] . enc[l, b]  +  (h[b] @ affect) . emb[l, b]
  out[b, 0, :] = softmax(e[b, :])

Strategy: data-parallel over batch (8 batches per core on 8 cores). The host
casts enc/emb/h to fp16 and pre-transposes enc so the hidden (contraction) dim
lies on SBUF partitions. The TensorEngine then computes the attention energies
directly: per (batch, 512-wide l-chunk) group, eight K=128 matmuls (stationary
hT[128,8], moving enc chunk [128,512]) plus one K=3 matmul for the affect term
(haT = affT @ hT computed on-device) accumulate f32 scores in PSUM. Each group
has its own 1MB DMA so the PE trails the stream by at most one group. The
softmax runs online: each batch's chunk-0 max is the exp bias for all its
chunks (f32 absorbs the range), so ScalarE's psum->SBUF copy is already the
exp pass and the tail is just a masked row-sum + reciprocal + normalize.
Engine APs need partition offsets % 32 == 0, so whole [8, 512] psum tiles land
in a staging tile (true scores in row b, cross-batch garbage elsewhere) and
per-batch DMAs (free partition addressing) gather the true rows. DMA
(fp16, ~33.6 MB/core) is the bottleneck; PE runs at ~65% occupancy underneath
it and DVE is nearly idle.
"""

import numpy as np

import concourse.bass as bass
import concourse.tile as tile
from concourse import bacc, mybir
from concourse.bass_utils import run_bass_kernel_spmd

F32 = mybir.dt.float32
F16 = mybir.dt.float16
L, B, H, A = 2048, 64, 1024, 3
NCORES = 8
BLOC = B // NCORES          # batches per core
P = 128                     # SBUF partitions
CH = 512                    # l-chunk width (one psum bank of f32)
NC_CH = L // CH             # chunks per batch (4)
NHO = H // P                # h-blocks (8)
NGRP = BLOC * NC_CH         # (b, c) groups per core (32)
GRPW = NHO * CH             # stream columns per group (4096)


def build_nc():
    nc = bacc.Bacc("TRN2", target_bir_lowering=False, debug=False)

    enc_d = nc.dram_tensor("enc", [P, NGRP * GRPW], F16, kind="ExternalInput")
    emb_d = nc.dram_tensor("emb", [A, NGRP * CH], F16, kind="ExternalInput")
    ht_d = nc.dram_tensor("ht", [P, NHO * BLOC], F16, kind="ExternalInput")
    afft_d = nc.dram_tensor("afft", [P, NHO * A], F16, kind="ExternalInput")
    out_d = nc.dram_tensor("out", [BLOC, L], F32, kind="ExternalOutput")

    add = mybir.AluOpType.add
    amax = mybir.AluOpType.max
    AX = mybir.AxisListType.X
    Exp = mybir.ActivationFunctionType.Exp

    with tile.TileContext(nc) as tc:
        with (
            tc.tile_pool(name="const", bufs=1) as cpool,
            tc.tile_pool(name="slab", bufs=5) as spool,
            tc.tile_pool(name="ps", bufs=7, space="PSUM") as ppool,
            tc.tile_pool(name="ps_ha", bufs=1, space="PSUM") as hpool,
        ):
            # small inputs on the gpsimd (SWDGE) queue; the big enc stream
            # owns the sync HWDGE queue from t=0
            ht = cpool.tile([P, NHO * BLOC], F16)
            nc.gpsimd.dma_start(ht[:], ht_d[:])
            afft = cpool.tile([P, NHO * A], F16)
            nc.gpsimd.dma_start(afft[:], afft_d[:])
            embt = cpool.tile([A, NGRP * CH], F16)
            nc.gpsimd.dma_start(embt[:], emb_d[:])

            # haT[a, b] = sum_h affect[h, a] * h[b, h]  (K-accumulate over
            # h-blocks; both operands arrive h-on-partitions)
            ha_ps = hpool.tile([A, BLOC], F32, tag="ha", name="ha_ps")
            for ho in range(NHO):
                nc.tensor.matmul(
                    ha_ps[:],
                    afft[:, ho * A:(ho + 1) * A],
                    ht[:, ho * BLOC:(ho + 1) * BLOC],
                    start=(ho == 0), stop=(ho == NHO - 1),
                )
            hat = cpool.tile([A, BLOC], F16)
            nc.vector.tensor_copy(hat[:], ha_ps[:])

            staging = cpool.tile([BLOC, NGRP * CH], F32)  # exp(e - m0[b])
            outstg = cpool.tile([BLOC, NGRP * CH], F32)   # normalized
            pm0 = cpool.tile([BLOC, BLOC], F32)           # chunk-0 maxes
            nm = cpool.tile([BLOC, BLOC], F32)            # -chunk-0 maxes
            cs = cpool.tile([BLOC, NGRP], F32)            # per-group exp sums
            sums = cpool.tile([BLOC, BLOC], F32)          # per-batch exp sums
            rc = cpool.tile([BLOC, BLOC], F32)            # reciprocals
            HG = GRPW // 2

            for g in range(NGRP):                         # g = b * NC_CH + c
                b, c = divmod(g, NC_CH)
                slab = spool.tile([P, GRPW], F16, tag="slab", name="slab")
                # half-group DMAs so the PE trails the stream by only
                # half a group; quarters for the last group to shrink the
                # end-of-stream PE lag further
                nparts = 8 if g == NGRP - 1 else (4 if g == NGRP - 2 else 2)
                pw = GRPW // nparts
                for q in range(nparts):
                    nc.sync.dma_start(
                        slab[:, q * pw:(q + 1) * pw],
                        enc_d[:, g * GRPW + q * pw:g * GRPW + (q + 1) * pw])
                ps = ppool.tile([BLOC, CH], F32, tag="ps", name="ps")
                nc.tensor.matmul(
                    ps[:], hat[:], embt[:, g * CH:(g + 1) * CH],
                    start=True, stop=False,
                )
                for ho in range(NHO):
                    nc.tensor.matmul(
                        ps[:],
                        ht[:, ho * BLOC:(ho + 1) * BLOC],
                        slab[:, ho * CH:(ho + 1) * CH],
                        start=False, stop=(ho == NHO - 1),
                    )
                if c == 0:
                    # batch b's exp bias for all four chunks; f32 absorbs
                    # exp(max_c - max_0) comfortably
                    nc.vector.tensor_reduce(pm0[:, b:b + 1], ps[:],
                                            axis=AX, op=amax)
                    nc.vector.tensor_scalar_mul(nm[:, b:b + 1],
                                                pm0[:, b:b + 1], -1.0)
                nc.scalar.activation(staging[:, g * CH:(g + 1) * CH], ps[:],
                                     Exp, bias=nm[:, b:b + 1], scale=1.0,
                                     accum_out=cs[:, g:g + 1])
                if c == NC_CH - 1:
                    # batch b complete: row-sum its 4 exp sums (foreign rows
                    # give garbage reciprocals applied only to garbage
                    # entries), normalize its staging columns split across
                    # DVE and ACT, and DMA the true row straight to HBM
                    nc.vector.tensor_reduce(
                        sums[:, b:b + 1], cs[:, b * NC_CH:(b + 1) * NC_CH],
                        axis=AX, op=add)
                    nc.vector.reciprocal(rc[:, b:b + 1], sums[:, b:b + 1])
                    lo = b * L
                    SPL = 1330        # DVE/ACT split (ACT has ~0.4us fixed overhead)
                    nc.vector.tensor_scalar_mul(
                        outstg[:, lo:lo + SPL],
                        staging[:, lo:lo + SPL], rc[:, b:b + 1])
                    nc.scalar.mul(
                        outstg[:, lo + SPL:lo + L],
                        staging[:, lo + SPL:lo + L], rc[:, b:b + 1])

            # output DMAs ride the sync queue after every slab DMA: they
            # never block the stream, and the SWDGE queue drains early; the
            # last batch's DMA is split so its first half transfers while
            # the ACT half of its normalize is still running
            SPL = 1330
            for b in range(BLOC):
                lo = b * L
                if b < BLOC - 1:
                    nc.sync.dma_start(out_d[b:b + 1, :],
                                      outstg[b:b + 1, lo:lo + L])
                else:
                    nc.sync.dma_start(out_d[b:b + 1, 0:SPL],
                                      outstg[b:b + 1, lo:lo + SPL])
                    nc.sync.dma_start(out_d[b:b + 1, SPL:L],
                                      outstg[b:b + 1, lo + SPL:lo + L])

    nc.compile()
    return nc


def make_in_maps(hidden, encoder_outputs, embedding, affect_matrix):
    aff16 = np.ascontiguousarray(affect_matrix, dtype=np.float16)
    # affT[k, ho*A + a] = affect[ho*128 + k, a]
    afft = np.ascontiguousarray(
        aff16.reshape(NHO, P, A).transpose(1, 0, 2).reshape(P, NHO * A))
    in_maps = []
    for i in range(NCORES):
        bs = slice(i * BLOC, (i + 1) * BLOC)
        enc16 = encoder_outputs[:, bs, :].astype(np.float16)  # [L, 8, H]
        # encT[k, (b, c, ho, j)] = enc[c*512 + j, b, ho*128 + k]
        enct = np.ascontiguousarray(
            enc16.reshape(NC_CH, CH, BLOC, NHO, P)
            .transpose(4, 2, 0, 3, 1).reshape(P, NGRP * GRPW))
        emb16 = embedding[:, bs, :].astype(np.float16)        # [L, 8, A]
        # embT[a, (b, c, j)] = emb[c*512 + j, b, a]
        embt = np.ascontiguousarray(
            emb16.reshape(NC_CH, CH, BLOC, A)
            .transpose(3, 2, 0, 1).reshape(A, NGRP * CH))
        h16 = hidden[0, bs, :].astype(np.float16)             # [8, H]
        # hT[k, ho*BLOC + b] = h[b, ho*128 + k]
        ht = np.ascontiguousarray(
            h16.reshape(BLOC, NHO, P).transpose(2, 1, 0).reshape(P, NHO * BLOC))
        in_maps.append({"enc": enct, "emb": embt, "ht": ht, "afft": afft})
    return in_maps


def assemble(results):
    return np.concatenate(
        [np.asarray(results[i]["out"], dtype=np.float32)[:, None, :]
         for i in range(NCORES)], axis=0)


_NC_CACHE = {}


def kernel(hidden, encoder_outputs, embedding, affect_matrix):
    hidden = np.asarray(hidden, dtype=np.float32)
    encoder_outputs = np.asarray(encoder_outputs, dtype=np.float32)
    embedding = np.asarray(embedding, dtype=np.float32)
    affect_matrix = np.asarray(affect_matrix, dtype=np.float32)

    if "nc" not in _NC_CACHE:
        _NC_CACHE["nc"] = build_nc()
    nc = _NC_CACHE["nc"]
    in_maps = make_in_maps(hidden, encoder_outputs, embedding, affect_matrix)
    res = run_bass_kernel_spmd(nc, in_maps, list(range(NCORES))).results
    return assemble(res)


# revision 30
# speedup vs baseline: 1.1111x; 1.1111x over previous
"""Bass/Trainium2 kernel for nn_Attn (dot+affect attention over encoder outputs).

Computation (per batch b):
  e[b, l] = h[b] . enc[l, b]  +  (h[b] @ affect) . emb[l, b]
  out[b, 0, :] = softmax(e[b, :])

Strategy: data-parallel over batch (8 batches per core on 8 cores). The host
casts enc/emb/h to fp16 and pre-transposes enc so the hidden (contraction) dim
lies on SBUF partitions. The TensorEngine then computes the attention energies
directly: per (batch, 512-wide l-chunk) group, eight K=128 matmuls (stationary
hT[128,8], moving enc chunk [128,512]) plus one K=3 matmul for the affect term
(haT = affT @ hT computed on-device) accumulate f32 scores in PSUM. Each group
has its own 1MB DMA so the PE trails the stream by at most one group. The
softmax runs online: each batch's chunk-0 max is the exp bias for all its
chunks (f32 absorbs the range), so ScalarE's psum->SBUF copy is already the
exp pass and the tail is just a masked row-sum + reciprocal + normalize.
Engine APs need partition offsets % 32 == 0, so whole [8, 512] psum tiles land
in a staging tile (true scores in row b, cross-batch garbage elsewhere) and
per-batch DMAs (free partition addressing) gather the true rows. DMA
(fp16, ~33.6 MB/core) is the bottleneck; PE runs at ~65% occupancy underneath
it and DVE is nearly idle.
"""

import numpy as np

import concourse.bass as bass
import concourse.tile as tile
from concourse import bacc, mybir
from concourse.bass_utils import run_bass_kernel_spmd

F32 = mybir.dt.float32
F16 = mybir.dt.float16
L, B, H, A = 2048, 64, 1024, 3
NCORES = 8
BLOC = B // NCORES          # batches per core
P = 128                     # SBUF partitions
CH = 512                    # l-chunk width (one psum bank of f32)
NC_CH = L // CH             # chunks per batch (4)
NHO = H // P                # h-blocks (8)
NGRP = BLOC * NC_CH         # (b, c) groups per core (32)
GRPW = NHO * CH             # stream columns per group (4096)


def build_nc():
    nc = bacc.Bacc("TRN2", target_bir_lowering=False, debug=False)

    enc_d = nc.dram_tensor("enc", [P, NGRP * GRPW], F16, kind="ExternalInput")
    emb_d = nc.dram_tensor("emb", [A, NGRP * CH], F16, kind="ExternalInput")
    ht_d = nc.dram_tensor("ht", [P, NHO * BLOC], F16, kind="ExternalInput")
    afft_d = nc.dram_tensor("afft", [P, NHO * A], F16, kind="ExternalInput")
    out_d = nc.dram_tensor("out", [BLOC, L], F32, kind="ExternalOutput")

    add = mybir.AluOpType.add
    amax = mybir.AluOpType.max
    AX = mybir.AxisListType.X
    Exp = mybir.ActivationFunctionType.Exp

    with tile.TileContext(nc) as tc:
        with (
            tc.tile_pool(name="const", bufs=1) as cpool,
            tc.tile_pool(name="slab", bufs=5) as spool,
            tc.tile_pool(name="ps", bufs=7, space="PSUM") as ppool,
            tc.tile_pool(name="ps_ha", bufs=1, space="PSUM") as hpool,
        ):
            # small inputs on the gpsimd (SWDGE) queue; the big enc stream
            # owns the sync HWDGE queue from t=0
            ht = cpool.tile([P, NHO * BLOC], F16)
            nc.gpsimd.dma_start(ht[:], ht_d[:])
            afft = cpool.tile([P, NHO * A], F16)
            nc.gpsimd.dma_start(afft[:], afft_d[:])
            embt = cpool.tile([A, NGRP * CH], F16)
            nc.gpsimd.dma_start(embt[:], emb_d[:])

            # haT[a, b] = sum_h affect[h, a] * h[b, h]  (K-accumulate over
            # h-blocks; both operands arrive h-on-partitions)
            ha_ps = hpool.tile([A, BLOC], F32, tag="ha", name="ha_ps")
            for ho in range(NHO):
                nc.tensor.matmul(
                    ha_ps[:],
                    afft[:, ho * A:(ho + 1) * A],
                    ht[:, ho * BLOC:(ho + 1) * BLOC],
                    start=(ho == 0), stop=(ho == NHO - 1),
                )
            hat = cpool.tile([A, BLOC], F16)
            nc.vector.tensor_copy(hat[:], ha_ps[:])

            staging = cpool.tile([BLOC, NGRP * CH], F32)  # exp(e - m0[b])
            outstg = cpool.tile([BLOC, NGRP * CH], F32)   # normalized
            pm0 = cpool.tile([BLOC, BLOC], F32)           # chunk-0 maxes
            nm = cpool.tile([BLOC, BLOC], F32)            # -chunk-0 maxes
            cs = cpool.tile([BLOC, NGRP], F32)            # per-group exp sums
            sums = cpool.tile([BLOC, BLOC], F32)          # per-batch exp sums
            rc = cpool.tile([BLOC, BLOC], F32)            # reciprocals
            HG = GRPW // 2

            for g in range(NGRP):                         # g = b * NC_CH + c
                b, c = divmod(g, NC_CH)
                slab = spool.tile([P, GRPW], F16, tag="slab", name="slab")
                # half-group DMAs so the PE trails the stream by only
                # half a group; quarters for the last group to shrink the
                # end-of-stream PE lag further
                nparts = 4 if g == NGRP - 1 else 2
                pw = GRPW // nparts
                for q in range(nparts):
                    nc.sync.dma_start(
                        slab[:, q * pw:(q + 1) * pw],
                        enc_d[:, g * GRPW + q * pw:g * GRPW + (q + 1) * pw])
                ps = ppool.tile([BLOC, CH], F32, tag="ps", name="ps")
                nc.tensor.matmul(
                    ps[:], hat[:], embt[:, g * CH:(g + 1) * CH],
                    start=True, stop=False,
                )
                for ho in range(NHO):
                    nc.tensor.matmul(
                        ps[:],
                        ht[:, ho * BLOC:(ho + 1) * BLOC],
                        slab[:, ho * CH:(ho + 1) * CH],
                        start=False, stop=(ho == NHO - 1),
                    )
                if c == 0:
                    # batch b's exp bias for all four chunks; f32 absorbs
                    # exp(max_c - max_0) comfortably
                    nc.vector.tensor_reduce(pm0[:, b:b + 1], ps[:],
                                            axis=AX, op=amax)
                    nc.vector.tensor_scalar_mul(nm[:, b:b + 1],
                                                pm0[:, b:b + 1], -1.0)
                nc.scalar.activation(staging[:, g * CH:(g + 1) * CH], ps[:],
                                     Exp, bias=nm[:, b:b + 1], scale=1.0,
                                     accum_out=cs[:, g:g + 1])
                if c == NC_CH - 1:
                    # batch b complete: row-sum its 4 exp sums (foreign rows
                    # give garbage reciprocals applied only to garbage
                    # entries), normalize its staging columns split across
                    # DVE and ACT, and DMA the true row straight to HBM
                    nc.vector.tensor_reduce(
                        sums[:, b:b + 1], cs[:, b * NC_CH:(b + 1) * NC_CH],
                        axis=AX, op=add)
                    nc.vector.reciprocal(rc[:, b:b + 1], sums[:, b:b + 1])
                    lo = b * L
                    SPL = 1330        # DVE/ACT split (ACT has ~0.4us fixed overhead)
                    nc.vector.tensor_scalar_mul(
                        outstg[:, lo:lo + SPL],
                        staging[:, lo:lo + SPL], rc[:, b:b + 1])
                    nc.scalar.mul(
                        outstg[:, lo + SPL:lo + L],
                        staging[:, lo + SPL:lo + L], rc[:, b:b + 1])

            # output DMAs ride the sync queue after every slab DMA: they
            # never block the stream, and the SWDGE queue drains early
            for b in range(BLOC):
                lo = b * L
                nc.sync.dma_start(out_d[b:b + 1, :],
                                  outstg[b:b + 1, lo:lo + L])

    nc.compile()
    return nc


def make_in_maps(hidden, encoder_outputs, embedding, affect_matrix):
    aff16 = np.ascontiguousarray(affect_matrix, dtype=np.float16)
    # affT[k, ho*A + a] = affect[ho*128 + k, a]
    afft = np.ascontiguousarray(
        aff16.reshape(NHO, P, A).transpose(1, 0, 2).reshape(P, NHO * A))
    in_maps = []
    for i in range(NCORES):
        bs = slice(i * BLOC, (i + 1) * BLOC)
        enc16 = encoder_outputs[:, bs, :].astype(np.float16)  # [L, 8, H]
        # encT[k, (b, c, ho, j)] = enc[c*512 + j, b, ho*128 + k]
        enct = np.ascontiguousarray(
            enc16.reshape(NC_CH, CH, BLOC, NHO, P)
            .transpose(4, 2, 0, 3, 1).reshape(P, NGRP * GRPW))
        emb16 = embedding[:, bs, :].astype(np.float16)        # [L, 8, A]
        # embT[a, (b, c, j)] = emb[c*512 + j, b, a]
        embt = np.ascontiguousarray(
            emb16.reshape(NC_CH, CH, BLOC, A)
            .transpose(3, 2, 0, 1).reshape(A, NGRP * CH))
        h16 = hidden[0, bs, :].astype(np.float16)             # [8, H]
        # hT[k, ho*BLOC + b] = h[b, ho*128 + k]
        ht = np.ascontiguousarray(
            h16.reshape(BLOC, NHO, P).transpose(2, 1, 0).reshape(P, NHO * BLOC))
        in_maps.append({"enc": enct, "emb": embt, "ht": ht, "afft": afft})
    return in_maps


def assemble(results):
    return np.concatenate(
        [np.asarray(results[i]["out"], dtype=np.float32)[:, None, :]
         for i in range(NCORES)], axis=0)


_NC_CACHE = {}


def kernel(hidden, encoder_outputs, embedding, affect_matrix):
    hidden = np.asarray(hidden, dtype=np.float32)
    encoder_outputs = np.asarray(encoder_outputs, dtype=np.float32)
    embedding = np.asarray(embedding, dtype=np.float32)
    affect_matrix = np.asarray(affect_matrix, dtype=np.float32)

    if "nc" not in _NC_CACHE:
        _NC_CACHE["nc"] = build_nc()
    nc = _NC_CACHE["nc"]
    in_maps = make_in_maps(hidden, encoder_outputs, embedding, affect_matrix)
    res = run_bass_kernel_spmd(nc, in_maps, list(range(NCORES))).results
    return assemble(res)
